# revision 13
# baseline (speedup 1.0000x reference)
"""ComplexAttention (B=2, T=2048, D=1024, H=16, Dh=64) on 8 TRN2 NeuronCores.

Sharding: core c -> batch b = c // 4, heads [4*(c%4), 4*(c%4)+4).
Each core computes its 4 heads' QKV projections (column-sharded), causal
complex attention, and a partial output projection (row-sharded). The four
partials per batch are summed ON DEVICE with per-token-quarter DRAM
ReduceScatters over the batch group (r/i fused into one [512, 2, D] tensor
per quarter, overlapped with the remaining attention compute), so each core
ships only a [T/4, 2, D] output slice — external output bytes per launch
dominate the measured time in this environment (~40us/MB), so 8MB -> 2MB
per core is the main win vs the host-summed variant. The host reassembles
slices and adds the (folded) output bias.

bf16 version (tolerance is 2e-2; bf16 matmuls stream 1 cyc/row vs 4 for
fp32 on the PE). Key tricks vs the fp32 baseline:
  - x is transposed AND cast to bf16 on the host: the device only ever needs
    x^T (Q/K rhs and V lhsT), so no PE transposes at all. DMAs are emitted
    in consumption order (wq_r, x_r-chunk0, wq_i, x_i-chunk0, ...) so the PE
    starts ~4us in.
  - Q/K are SBUF-resident [128, T] per head in "complex-stacked" layout:
    even head h: [qr_h(64) ; qi_h(64)], odd head h: [qi_h(64) ; qr_h(64)].
    Score contraction qr.kr + qi.ki is order-invariant, and this swap makes
    pair-packed M=128 projection matmuls land partition-aligned:
      psA = [re_h0 ; re_h1] (real weights, natural pair order)
      psB = [im_h1 ; im_h0] (imag weights, swapped pair order)
  - attn@V is ONE M=128 matmul per k-tile: v_sb head block is [vr|vi] for
    even heads, [vi|vr] for odd heads, so po rows split directly into the
    ort/oit pair layouts ([vr_even;vr_odd] / [vi_odd;vi_even]).
  - V bias is folded out entirely: post-softmax weights sum to 1, so
    A(XWv + bv) = A X Wv + bv; the host adds bv@Wo to the output bias.
  - Causal mask: 4 precomputed [128,512] bf16 mask tiles (one per diagonal
    offset), applied as a DVE multiply on exp(S^T) tiles.
  - Phase 2 runs j-outer/head-inner and the phase-3 output projection for
    token chunk j is emitted right after, so it pipelines into attention;
    its PSUM-to-bf16 cast copies rotate over ACT/DVE.
  - Phase-2 PE stream is software-pipelined (st_{k+1} emitted before po_k)
    so the PE never waits on the ACT exp round trip.
Attention math identical to baseline: S^T tiles on PE, exp on ACT (no max
subtraction: |S| <~ 4), l via ones-matmul, 1/l broadcast via K=1 matmul.
"""

import math
from contextlib import ExitStack

import numpy as np
import ml_dtypes

import concourse.bass as bass
import concourse.tile as tile
from concourse import bacc, mybir
from concourse.bass_utils import run_bass_kernel_spmd

F32 = mybir.dt.float32
BF16 = mybir.dt.bfloat16
NP_BF16 = ml_dtypes.bfloat16

# Full-problem config (hardcoded per harness contract).
CFG = dict(T=2048, D=1024, HPC=4, DH=64, TCH=512, QCH=512)
N_CORES = 8
B = 2
H_TOTAL = 16

# Flipped by test.py for profiling; harness path keeps these defaults.
TRACE = False
LAST = {}


def build_program(cfg, num_devices=N_CORES, enable_asserts=False):
    """Build the per-core SPMD Bass program. Returns nc."""
    T, D, HPC, DH = cfg["T"], cfg["D"], cfg["HPC"], cfg["DH"]
    TCH, QCH = cfg["TCH"], cfg["QCH"]
    P = 128
    DT = D // P            # din tiles
    NCH = T // TCH         # phase-1 token chunks
    KT = T // P            # key tiles
    QC = T // QCH          # phase-2 query chunks
    QKB = QCH // P         # key tiles per query chunk step
    CW = HPC * DH          # per-core qkv width
    NPAIR = HPC // 2
    scale = 1.0 / math.sqrt(DH)

    assert DH == 64 and P == 128 and CW % 128 == 0

    nc = bacc.Bacc(
        "TRN2",
        target_bir_lowering=False,
        debug=False,
        enable_asserts=enable_asserts,
        num_devices=num_devices,
    )

    # ---- DRAM I/O (x ships pre-transposed d-major, everything bf16) ----
    xt_r = nc.dram_tensor("xt_r", [D, T], BF16, kind="ExternalInput").ap()
    xt_i = nc.dram_tensor("xt_i", [D, T], BF16, kind="ExternalInput").ap()
    wq_r = nc.dram_tensor("wq_r", [D, CW], BF16, kind="ExternalInput").ap()
    wq_i = nc.dram_tensor("wq_i", [D, CW], BF16, kind="ExternalInput").ap()
    wk_r = nc.dram_tensor("wk_r", [D, CW], BF16, kind="ExternalInput").ap()
    wk_i = nc.dram_tensor("wk_i", [D, CW], BF16, kind="ExternalInput").ap()
    wv_r = nc.dram_tensor("wv_r", [D, CW], BF16, kind="ExternalInput").ap()
    wv_i = nc.dram_tensor("wv_i", [D, CW], BF16, kind="ExternalInput").ap()
    wo_r = nc.dram_tensor("wo_r", [CW, D], BF16, kind="ExternalInput").ap()
    wo_i = nc.dram_tensor("wo_i", [CW, D], BF16, kind="ExternalInput").ap()
    bq = nc.dram_tensor("bq", [P, HPC], F32, kind="ExternalInput").ap()
    bk = nc.dram_tensor("bk", [P, HPC], F32, kind="ExternalInput").ap()
    # Partials stay on-device: internal [T, 2, D] (r/i fused per token),
    # reduce-scattered per token-quarter over the 4-core batch group; each
    # core ships only its [T/4, 2, D] slice.
    par = nc.dram_tensor("par", [T, 2, D], BF16).ap()
    red = nc.dram_tensor("red", [T // 4, 2, D], BF16).ap()
    out = nc.dram_tensor("out", [T // 4, 2, D], BF16, kind="ExternalOutput").ap()

    xt_r_t = xt_r.rearrange("(n p) t -> p n t", p=P)
    xt_i_t = xt_i.rearrange("(n p) t -> p n t", p=P)
    par_t = par.rearrange("(n p) two d -> p n two d", p=P)
    rs_groups = [[0, 1, 2, 3], [4, 5, 6, 7]]

    def reduce_quarter(j):
        """ReduceScatter token-quarter j (r+i fused) across the batch group
        and stage this core's T/16 slice of it to the external output."""
        rows = slice(j * QCH, (j + 1) * QCH)
        orow = slice(j * (QCH // 4), (j + 1) * (QCH // 4))
        nc.gpsimd.collective_compute(
            "ReduceScatter", mybir.AluOpType.add, replica_groups=rs_groups,
            ins=[par[rows].opt()], outs=[red[orow].opt()])
        nc.sync.dma_start(out[orow], red[orow])

    with tile.TileContext(nc) as tc, ExitStack() as octx:
        # ---- long-lived pools ----
        const = octx.enter_context(tc.tile_pool(name="const", bufs=1))
        opool = octx.enter_context(tc.tile_pool(name="opool", bufs=1))
        wpool = octx.enter_context(tc.tile_pool(name="wpool", bufs=1))

        # x^T resident [p, din_tile, T]; weights [p, din_tile, CW].
        # DMAs emitted in first-use order so phase 1 starts early: the very
        # first matmul needs only wq_r and x_r chunk 0.
        xrT = opool.tile([P, DT, T], BF16, name="xrT")
        xiT = opool.tile([P, DT, T], BF16, name="xiT")

        def load_w(ap_dram, name):
            w = wpool.tile([P, DT, CW], BF16, name=name)
            nc.sync.dma_start(w, ap_dram.rearrange("(t p) m -> p t m", p=P))
            return w

        def load_x_chunk(tch, which="ri"):
            cs = slice(tch * TCH, (tch + 1) * TCH)
            if "r" in which:
                nc.sync.dma_start(xrT[:, :, cs], xt_r_t[:, :, cs])
            if "i" in which:
                nc.sync.dma_start(xiT[:, :, cs], xt_i_t[:, :, cs])

        wq_r_sb = load_w(wq_r, "wq_r_sb")
        load_x_chunk(0, "r")
        wq_i_sb = load_w(wq_i, "wq_i_sb")
        load_x_chunk(0, "i")
        bq_sb = const.tile([P, HPC], F32)
        nc.sync.dma_start(bq_sb, bq)
        bk_sb = const.tile([P, HPC], F32)
        nc.sync.dma_start(bk_sb, bk)
        wk_r_sb = load_w(wk_r, "wk_r_sb")
        wk_i_sb = load_w(wk_i, "wk_i_sb")
        load_x_chunk(1)
        wv_r_sb = load_w(wv_r, "wv_r_sb")
        wv_i_sb = load_w(wv_i, "wv_i_sb")
        for tch in range(2, NCH):
            load_x_chunk(tch)

        ones_st = const.tile([P, P], F32)
        nc.vector.memset(ones_st, 1.0)
        ones_col = const.tile([P, 1], BF16)   # lhsT for l = ones^T @ expS
        nc.scalar.activation(ones_col, ones_st[:, 0:1],
                             mybir.ActivationFunctionType.Copy)
        ones_row = const.tile([1, P], BF16)   # lhsT for K=1 broadcasts
        nc.scalar.activation(ones_row, ones_st[0:1, :],
                             mybir.ActivationFunctionType.Copy)
        # Causal mask tiles: mask[o][p, f] = (f - p >= 128*o), o = k - 4j
        masks = const.tile([P, QKB, QCH], BF16)
        nc.vector.memset(masks, 1.0)
        for o in range(QKB):
            nc.gpsimd.affine_select(
                out=masks[:, o, :], in_=masks[:, o, :],
                compare_op=mybir.AluOpType.is_ge, fill=0.0,
                base=-P * o, pattern=[[1, QCH]], channel_multiplier=-1)

        # Q/K resident, complex-stacked per head (odd heads swapped)
        qh = [opool.tile([P, T], BF16, name=f"qh{h}") for h in range(HPC)]
        kh = [opool.tile([P, T], BF16, name=f"kh{h}") for h in range(HPC)]
        # V resident: [p, ktile, head*128 + (64|64)]; even head [vr|vi],
        # odd head [vi|vr]
        v_sb = opool.tile([P, KT, HPC * P], BF16)
        # O^T head-pair blocks, SBUF-resident into phase 3.
        # ORT[pair] rows: [vr_h_even(64) ; vr_h_odd(64)]
        # OIT[pair] rows: [vi_h_odd(64) ; vi_h_even(64)]  (host permutes wo_i)
        ort = [opool.tile([P, T], BF16, name=f"ort{p}") for p in range(NPAIR)]
        oit = [opool.tile([P, T], BF16, name=f"oit{p}") for p in range(NPAIR)]

        # ================= Phase 1: projections =================
        with ExitStack() as ctx:
            ps_qk = ctx.enter_context(tc.tile_pool(name="ps_qk", bufs=4, space="PSUM"))

            # Q/K pair-packed: psA = [re_h0;re_h1], psB = [im_h1;im_h0]
            def qk_chunk(tch):
                cs = slice(tch * TCH, (tch + 1) * TCH)
                for pr in range(NPAIR):
                    h0, h1 = 2 * pr, 2 * pr + 1
                    mA = slice(pr * P, pr * P + P)
                    for (wr, wi, bias, dst) in (
                        (wq_r_sb, wq_i_sb, bq_sb, qh),
                        (wk_r_sb, wk_i_sb, bk_sb, kh),
                    ):
                        psA = ps_qk.tile([P, TCH], F32, name="psA", tag="psqk")
                        psB = ps_qk.tile([P, TCH], F32, name="psB", tag="psqk")
                        for d in range(DT):
                            nc.tensor.matmul(
                                psA, wr[:, d, mA], xrT[:, d, cs],
                                start=(d == 0), stop=(d == DT - 1))
                            # host swapped the imag pair columns -> [h1, h0]
                            nc.tensor.matmul(
                                psB, wi[:, d, mA], xiT[:, d, cs],
                                start=(d == 0), stop=(d == DT - 1))
                        nc.vector.tensor_scalar_add(
                            out=dst[h0][0:64, cs], in0=psA[0:64],
                            scalar1=bias[0:64, h0:h0 + 1])
                        nc.vector.tensor_scalar_add(
                            out=dst[h1][64:128, cs], in0=psA[64:128],
                            scalar1=bias[64:128, h1:h1 + 1])
                        nc.vector.tensor_scalar_add(
                            out=dst[h1][0:64, cs], in0=psB[0:64],
                            scalar1=bias[0:64, h1:h1 + 1])
                        nc.vector.tensor_scalar_add(
                            out=dst[h0][64:128, cs], in0=psB[64:128],
                            scalar1=bias[64:128, h0:h0 + 1])

            # V token-major: psum [tok(128), CW] for r and i, then pack into
            # v_sb[:, kt, head*128 + ...] with odd-head [vi|vr] swap.
            # Pack copies go on ACT (idle during phase 1 otherwise).
            ps_v1 = ctx.enter_context(tc.tile_pool(name="ps_v1", bufs=4, space="PSUM"))

            def v_ktile(ktile):
                t0 = ktile * P
                pvr = ps_v1.tile([P, CW], F32, name="pvr", tag="pv")
                for d in range(DT):
                    nc.tensor.matmul(
                        pvr, xrT[:, d, t0:t0 + P], wv_r_sb[:, d, :],
                        start=(d == 0), stop=(d == DT - 1))
                pvi = ps_v1.tile([P, CW], F32, name="pvi", tag="pv")
                for d in range(DT):
                    nc.tensor.matmul(
                        pvi, xiT[:, d, t0:t0 + P], wv_i_sb[:, d, :],
                        start=(d == 0), stop=(d == DT - 1))
                for h in range(HPC):
                    lo = h % 2
                    dst_r = v_sb[:, ktile,
                                 h * P + 64 * lo:h * P + 64 * lo + 64]
                    dst_i = v_sb[:, ktile,
                                 h * P + 64 * (1 - lo):h * P + 64 * (1 - lo) + 64]
                    nc.scalar.copy(out=dst_r, in_=pvr[:, h * DH:(h + 1) * DH])
                    nc.scalar.copy(out=dst_i, in_=pvi[:, h * DH:(h + 1) * DH])

            # Interleave: V matmul stretches give the DVE add queue time to
            # drain before phase 2 consumes the last chunks' Q/K.
            qk_chunk(0)
            qk_chunk(1)
            for kt_ in range(0, KT // 2):
                v_ktile(kt_)
            qk_chunk(2)
            qk_chunk(3)
            for kt_ in range(KT // 2, KT):
                v_ktile(kt_)

        # ========== Phase 2+3: causal attention + output projection ==========
        # j-outer: after all 4 heads finish query-chunk j, the phase-3
        # projection for that chunk's 4 token tiles is emitted.
        with ExitStack() as ctx:
            epool = ctx.enter_context(tc.tile_pool(name="epool", bufs=12))
            rpool = ctx.enter_context(tc.tile_pool(name="rpool", bufs=2))
            sout = ctx.enter_context(tc.tile_pool(name="sout", bufs=6))
            # 8 PSUM banks: big(st,pb,pf) 3 + po 3 + pl 2
            ps_big = ctx.enter_context(tc.tile_pool(name="ps_big", bufs=3, space="PSUM"))
            ps_o = ctx.enter_context(tc.tile_pool(name="ps_o", bufs=3, space="PSUM"))
            ps_l = ctx.enter_context(tc.tile_pool(name="ps_l", bufs=2, space="PSUM"))

            wor_sb = wpool.tile([P, NPAIR, D], BF16, name="wor_sb")
            nc.sync.dma_start(
                wor_sb, wo_r.rearrange("(t p) m -> p t m", p=P))
            woi_sb = wpool.tile([P, NPAIR, D], BF16, name="woi_sb")
            nc.sync.dma_start(
                woi_sb, wo_i.rearrange("(t p) m -> p t m", p=P))

            NC2 = D // 512
            flip = 0

            def phase3_chunk(j, parts=(0, 1)):
                nonlocal flip
                tensors = ((ort, wor_sb, 0), (oit, woi_sb, 1))
                for t in range(j * QKB, (j + 1) * QKB):
                    for (oblocks, wsb, ri) in (tensors[p] for p in parts):
                        for n in range(NC2):
                            pf = ps_big.tile([P, 512], F32, name="pf",
                                             tag="big")
                            for kk in range(NPAIR):
                                nc.tensor.matmul(
                                    pf,
                                    oblocks[kk][:, t * P:(t + 1) * P],
                                    wsb[:, kk, n * 512:(n + 1) * 512],
                                    start=(kk == 0), stop=(kk == NPAIR - 1))
                            ot = sout.tile([P, 512], BF16, name="ot")
                            # alternate the cast-copy between ACT and DVE
                            if flip % 2 == 0:
                                nc.scalar.copy(out=ot, in_=pf)
                            else:
                                nc.vector.tensor_copy(out=ot, in_=pf)
                            flip += 1
                            nc.sync.dma_start(
                                par_t[:, t, ri, n * 512:(n + 1) * 512], ot)

            # Flat block list; the score->exp->mask stage cursor runs two
            # k-steps ahead GLOBALLY (crossing block boundaries), so the PE
            # never waits for the ACT round trip, even at block starts.
            blocks = [(j, pr) for j in range(QC) for pr in range(NPAIR)]
            ets = {}

            def stage(bi, k):
                j, pr = blocks[bi]
                # Diagonal blocks (o >= 1): columns f < 128*o are fully
                # masked -> compute st/exp/mask/po on [128*o, QCH) only.
                o = k - j * QKB
                off = max(0, o) * P if o >= 1 else 0
                qs = slice(j * QCH + off, (j + 1) * QCH)
                for h in (2 * pr, 2 * pr + 1):
                    st = ps_big.tile([P, QCH], F32, name="st", tag="big")
                    nc.tensor.matmul(
                        st[:, off:], kh[h][:, k * P:(k + 1) * P],
                        qh[h][:, qs], start=True, stop=True)
                    et = epool.tile([P, QCH], BF16, name="et")
                    nc.scalar.activation(
                        et[:, off:], st[:, off:],
                        mybir.ActivationFunctionType.Exp, scale=scale)
                    if o >= 0:
                        nc.vector.tensor_mul(
                            out=et[:, off:], in0=et[:, off:],
                            in1=masks[:, o, off:])
                    ets[bi, h, k] = et

            cursor = [0, 0]

            def emit_next_stage():
                bi, k = cursor
                if bi >= len(blocks):
                    return
                stage(bi, k)
                if k + 1 < (blocks[bi][0] + 1) * QKB:
                    cursor[:] = [bi, k + 1]
                else:
                    cursor[:] = [bi + 1, 0]

            for _ in range(5):
                emit_next_stage()
            for bi, (j, pr) in enumerate(blocks):
                nk = (j + 1) * QKB
                qs = slice(j * QCH, (j + 1) * QCH)
                hs = (2 * pr, 2 * pr + 1)

                # Dependency-free phase-3 PE filler at block transitions
                # (chunk j's ort/oit complete after block (j, pair1)); the
                # finished quarter's ReduceScatter is kicked off right after
                # its last partial DMA is emitted.
                def fill_ri(jc):
                    def f():
                        phase3_chunk(jc, parts=(1,))
                        reduce_quarter(jc)
                    return f

                filler = {2: lambda: phase3_chunk(0, parts=(0,)),
                          3: fill_ri(0),
                          4: lambda: phase3_chunk(1, parts=(0,)),
                          5: fill_ri(1),
                          6: lambda: phase3_chunk(2, parts=(0,)),
                          7: fill_ri(2)}.get(bi)
                if filler is not None:
                    filler()
                po = {h: ps_o.tile([P, QCH], F32, name=f"po{h}", tag="po")
                      for h in hs}
                pl = {h: ps_l.tile([1, QCH], F32, name=f"pl{h}", tag="pl")
                      for h in hs}
                for k in range(nk):
                    emit_next_stage()
                    o = k - j * QKB
                    off = max(0, o) * P if o >= 1 else 0
                    for h in hs:
                        et = ets.pop((bi, h, k))
                        nc.tensor.matmul(
                            pl[h][:, off:], ones_col, et[:, off:],
                            start=(k == 0), stop=(k == nk - 1))
                        # po rows: even head [o_r;o_i], odd [o_i;o_r]
                        nc.tensor.matmul(
                            po[h][:, off:], v_sb[:, k, h * P:(h + 1) * P],
                            et[:, off:],
                            start=(k == 0), stop=(k == nk - 1))

                rls = {}
                for h in hs:
                    rl = rpool.tile([1, QCH], BF16, name="rl")
                    with nc.allow_low_precision(
                            reason="1/l in bf16 feeds bf16 bcast matmul"):
                        nc.vector.reciprocal(rl, pl[h])
                    rls[h] = rl
                for h in hs:
                    lo = h % 2
                    base_r = 64 * lo          # vr rows in ORT[pair]
                    base_i = 64 * (1 - lo)    # vi rows in OIT[pair]
                    pb = ps_big.tile([P, QCH], F32, name="pb", tag="big")
                    nc.tensor.matmul(pb, ones_row, rls[h],
                                     start=True, stop=True)
                    sb_b = rpool.tile([P, QCH], BF16, name="sb_b")
                    nc.vector.tensor_copy(out=sb_b, in_=pb)
                    nc.vector.tensor_mul(
                        out=ort[pr][base_r:base_r + 64, qs],
                        in0=po[h][64 * lo:64 * lo + 64],
                        in1=sb_b[64 * lo:64 * lo + 64])
                    nc.vector.tensor_mul(
                        out=oit[pr][base_i:base_i + 64, qs],
                        in0=po[h][64 * (1 - lo):64 * (1 - lo) + 64],
                        in1=sb_b[64 * (1 - lo):64 * (1 - lo) + 64])
            phase3_chunk(QC - 1)
            reduce_quarter(QC - 1)

    nc.compile()
    return nc


def make_core_inputs(inputs, cfg=CFG):
    """Slice full inputs into 8 per-core input maps (bf16, pre-transposed x,
    pair-swapped imag weight columns, odd-head-swapped biases)."""
    HPC, DH = cfg["HPC"], cfg["DH"]
    CW = HPC * DH
    NPAIR = HPC // 2
    f32 = lambda a: np.asarray(a, dtype=np.float32)
    bf = lambda a: np.ascontiguousarray(np.asarray(a, np.float32)).astype(NP_BF16)

    def pair_swap_cols(w):
        # [D, CW]: per pair swap the two head column blocks
        out = np.empty_like(w)
        for p in range(NPAIR):
            out[:, p * 2 * DH:p * 2 * DH + DH] = \
                w[:, p * 2 * DH + DH:p * 2 * DH + 2 * DH]
            out[:, p * 2 * DH + DH:p * 2 * DH + 2 * DH] = \
                w[:, p * 2 * DH:p * 2 * DH + DH]
        return out

    x_real, x_imag = f32(inputs["x_real"]), f32(inputs["x_imag"])
    maps = []
    for c in range(N_CORES):
        b = c // 4
        g = c % 4
        cs = slice(g * CW, (g + 1) * CW)
        bqr, bqi = f32(inputs["bqr"])[cs], f32(inputs["bqi"])[cs]
        bkr, bki = f32(inputs["bkr"])[cs], f32(inputs["bki"])[cs]

        def head_bias(br, bi):
            # col h: even [br_h; bi_h], odd [bi_h; br_h]
            cols = []
            for h in range(HPC):
                r = br[h * DH:(h + 1) * DH]
                i = bi[h * DH:(h + 1) * DH]
                cols.append(np.concatenate([r, i] if h % 2 == 0 else [i, r]))
            return np.ascontiguousarray(np.stack(cols, axis=1))

        woi = f32(inputs["Woi"])[cs, :]
        # OIT pair rows are [h_odd ; h_even] -> permute wo_i rows to match
        woi_perm = np.concatenate(
            [np.concatenate([woi[2 * p * DH + DH:2 * p * DH + 2 * DH],
                             woi[2 * p * DH:2 * p * DH + DH]])
             for p in range(NPAIR)])
        maps.append({
            "xt_r": bf(x_real[b].T), "xt_i": bf(x_imag[b].T),
            "wq_r": bf(f32(inputs["Wqr"])[:, cs]),
            "wq_i": bf(pair_swap_cols(f32(inputs["Wqi"])[:, cs])),
            "wk_r": bf(f32(inputs["Wkr"])[:, cs]),
            "wk_i": bf(pair_swap_cols(f32(inputs["Wki"])[:, cs])),
            "wv_r": bf(f32(inputs["Wvr"])[:, cs]),
            "wv_i": bf(f32(inputs["Wvi"])[:, cs]),
            "wo_r": bf(f32(inputs["Wor"])[cs, :]), "wo_i": bf(woi_perm),
            "bq": head_bias(bqr, bqi), "bk": head_bias(bkr, bki),
        })
    return maps


def effective_out_bias(inputs):
    """V bias folded through the output projection (softmax weights sum
    to 1): bor_eff = bor + bvr @ Wor, boi_eff = boi + bvi @ Woi."""
    f = lambda a: np.asarray(a, dtype=np.float32)
    bor = f(inputs["bor"]) + f(inputs["bvr"]) @ f(inputs["Wor"])
    boi = f(inputs["boi"]) + f(inputs["bvi"]) @ f(inputs["Woi"])
    return bor, boi


def assemble_output(results, inputs):
    """Assemble full outputs from per-core reduce-scattered [T/4, 2, D]
    slices. Core 4b+r holds, for each token-quarter j, reduced token rows
    [512j+128r, 512j+128(r+1)) at local rows [128j, 128(j+1))."""
    T, D = CFG["T"], CFG["D"]
    QCH = CFG["QCH"]
    bor, boi = effective_out_bias(inputs)
    full_r = np.empty((B, T, D), np.float32)
    full_i = np.empty((B, T, D), np.float32)
    for b in range(B):
        for r in range(4):
            sl = np.asarray(results[4 * b + r]["out"], dtype=np.float32)
            for j in range(T // QCH):
                rows = slice(QCH * j + 128 * r, QCH * j + 128 * (r + 1))
                full_r[b, rows] = sl[128 * j:128 * (j + 1), 0]
                full_i[b, rows] = sl[128 * j:128 * (j + 1), 1]
        full_r[b] += bor
        full_i[b] += boi
    return full_r, full_i


def kernel(**inputs):
    global LAST
    nc = build_program(CFG)
    in_maps = make_core_inputs(inputs)
    res = run_bass_kernel_spmd(
        nc, in_maps, core_ids=list(range(N_CORES)), trace=TRACE)
    LAST = {"exec_time_ns": res.exec_time_ns,
            "trace": res.instructions_and_trace,
            "profile_json": res.profile_json,
            "nc": nc}
    results = [{k: np.asarray(v) for k, v in res.results[c].items()}
               for c in range(N_CORES)]
    return assemble_output(results, inputs)


# revision 14
# speedup vs baseline: 1.5278x; 1.5278x over previous
"""ComplexAttention (B=2, T=2048, D=1024, H=16, Dh=64) on 8 TRN2 NeuronCores.

Sharding: core c -> batch b = c // 4, heads [4*(c%4), 4*(c%4)+4).
Each core computes its 4 heads' QKV projections (column-sharded), causal
complex attention, and a partial output projection (row-sharded). The host
sums the 4 partials per batch and adds the (folded) output bias.

I/O note: in this environment the measured per-launch time is dominated by
per-BUFFER marshalling cost (~30-50us per bound tensor), not by kernel
compute or DMA bytes. So ALL 12 inputs are packed into ONE host-prelaid
bf16 blob in exact SBUF layout ([128, cols], sliced by column offsets), and
the two [T, D] partial outputs are fused into ONE [T, 2, D] tensor.

bf16 version (tolerance is 2e-2; bf16 matmuls stream 1 cyc/row vs 4 for
fp32 on the PE). Key tricks vs the fp32 baseline:
  - x is transposed AND cast to bf16 on the host: the device only ever needs
    x^T (Q/K rhs and V lhsT), so no PE transposes at all. DMAs are emitted
    in consumption order (wq_r, x_r-chunk0, wq_i, x_i-chunk0, ...) so the PE
    starts ~4us in.
  - Q/K are SBUF-resident [128, T] per head in "complex-stacked" layout:
    even head h: [qr_h(64) ; qi_h(64)], odd head h: [qi_h(64) ; qr_h(64)].
    Score contraction qr.kr + qi.ki is order-invariant, and this swap makes
    pair-packed M=128 projection matmuls land partition-aligned:
      psA = [re_h0 ; re_h1] (real weights, natural pair order)
      psB = [im_h1 ; im_h0] (imag weights, swapped pair order)
  - attn@V is ONE M=128 matmul per k-tile: v_sb head block is [vr|vi] for
    even heads, [vi|vr] for odd heads, so po rows split directly into the
    ort/oit pair layouts ([vr_even;vr_odd] / [vi_odd;vi_even]).
  - V bias is folded out entirely: post-softmax weights sum to 1, so
    A(XWv + bv) = A X Wv + bv; the host adds bv@Wo to the output bias.
  - Causal mask: 4 precomputed [128,512] bf16 mask tiles (one per diagonal
    offset), applied as a DVE multiply on exp(S^T) tiles.
  - Phase 2 runs j-outer/head-inner and the phase-3 output projection for
    token chunk j is emitted right after, so it pipelines into attention;
    its PSUM-to-bf16 cast copies rotate over ACT/DVE.
  - Phase-2 PE stream is software-pipelined (st_{k+1} emitted before
    pl_k/po_k) so the PE never waits on the ACT exp round trip.
Attention math identical to baseline: S^T tiles on PE, exp on ACT (no max
subtraction: |S| <~ 4), l via ones-matmul, 1/l broadcast via K=1 matmul.
"""

import math
from contextlib import ExitStack

import numpy as np
import ml_dtypes

import concourse.bass as bass
import concourse.tile as tile
from concourse import bacc, mybir
from concourse.bass_utils import run_bass_kernel_spmd

F32 = mybir.dt.float32
BF16 = mybir.dt.bfloat16
NP_BF16 = ml_dtypes.bfloat16

# Full-problem config (hardcoded per harness contract).
CFG = dict(T=2048, D=1024, HPC=4, DH=64, TCH=512, QCH=512)
N_CORES = 8
B = 2
H_TOTAL = 16

# Flipped by test.py for profiling; harness path keeps these defaults.
TRACE = False
LAST = {}

P = 128


def _blob_offsets(cfg):
    """Column offsets of each logical tensor inside the [128, cols] blob."""
    T, D, HPC, DH = cfg["T"], cfg["D"], cfg["HPC"], cfg["DH"]
    CW = HPC * DH
    DT = D // P
    NPAIR = HPC // 2
    sizes = [
        ("xt_r", DT * T), ("xt_i", DT * T),
        ("wq_r", DT * CW), ("wq_i", DT * CW),
        ("wk_r", DT * CW), ("wk_i", DT * CW),
        ("wv_r", DT * CW), ("wv_i", DT * CW),
        ("wo_r", NPAIR * D), ("wo_i", NPAIR * D),
        ("bq", HPC), ("bk", HPC),
    ]
    offs, o = {}, 0
    for nm, sz in sizes:
        offs[nm] = (o, sz)
        o += sz
    return offs, o


def build_program(cfg, num_devices=N_CORES, enable_asserts=False):
    """Build the per-core SPMD Bass program. Returns nc."""
    T, D, HPC, DH = cfg["T"], cfg["D"], cfg["HPC"], cfg["DH"]
    TCH, QCH = cfg["TCH"], cfg["QCH"]
    DT = D // P            # din tiles
    NCH = T // TCH         # phase-1 token chunks
    KT = T // P            # key tiles
    QC = T // QCH          # phase-2 query chunks
    QKB = QCH // P         # key tiles per query chunk step
    CW = HPC * DH          # per-core qkv width
    NPAIR = HPC // 2
    scale = 1.0 / math.sqrt(DH)

    assert DH == 64 and P == 128 and CW % 128 == 0

    nc = bacc.Bacc(
        "TRN2",
        target_bir_lowering=False,
        debug=False,
        enable_asserts=enable_asserts,
        num_devices=num_devices,
    )

    # ---- DRAM I/O: ONE bf16 input blob in SBUF layout + ONE fused output
    offs, cols = _blob_offsets(cfg)
    blob = nc.dram_tensor("blob", [P, cols], BF16, kind="ExternalInput").ap()
    out = nc.dram_tensor("out", [T, 2, D], BF16, kind="ExternalOutput").ap()
    out_t = out.rearrange("(n p) two d -> p n two d", p=P)

    def bslice(nm, shape=None):
        o, sz = offs[nm]
        ap = blob[:, o:o + sz]
        if shape is not None:
            ap = ap.rearrange("p (n m) -> p n m", m=shape[-1])
        return ap

    with tile.TileContext(nc) as tc, ExitStack() as octx:
        # ---- long-lived pools ----
        const = octx.enter_context(tc.tile_pool(name="const", bufs=1))
        opool = octx.enter_context(tc.tile_pool(name="opool", bufs=1))
        wpool = octx.enter_context(tc.tile_pool(name="wpool", bufs=1))

        # x^T resident [p, din_tile, T]; weights [p, din_tile, CW].
        # DMAs emitted in first-use order so phase 1 starts early: the very
        # first matmul needs only wq_r and x_r chunk 0.
        xrT = opool.tile([P, DT, T], BF16, name="xrT")
        xiT = opool.tile([P, DT, T], BF16, name="xiT")
        x_src = {"r": bslice("xt_r", (DT, T)), "i": bslice("xt_i", (DT, T))}

        def load_w(nm):
            w = wpool.tile([P, DT, CW], BF16, name=nm + "_sb")
            nc.sync.dma_start(w, bslice(nm, (DT, CW)))
            return w

        def load_x_chunk(tch, which="ri"):
            cs = slice(tch * TCH, (tch + 1) * TCH)
            if "r" in which:
                nc.sync.dma_start(xrT[:, :, cs], x_src["r"][:, :, cs])
            if "i" in which:
                nc.sync.dma_start(xiT[:, :, cs], x_src["i"][:, :, cs])

        wq_r_sb = load_w("wq_r")
        load_x_chunk(0, "r")
        wq_i_sb = load_w("wq_i")
        load_x_chunk(0, "i")
        # biases arrive bf16 in the blob; upcast once to f32 scalars
        bqk_bf = const.tile([P, 2, HPC], BF16)
        nc.sync.dma_start(bqk_bf[:, 0, :], bslice("bq"))
        nc.sync.dma_start(bqk_bf[:, 1, :], bslice("bk"))
        bqk = const.tile([P, 2, HPC], F32)
        nc.scalar.copy(out=bqk, in_=bqk_bf)
        bq_sb, bk_sb = bqk[:, 0, :], bqk[:, 1, :]
        wk_r_sb = load_w("wk_r")
        wk_i_sb = load_w("wk_i")
        load_x_chunk(1)
        wv_r_sb = load_w("wv_r")
        wv_i_sb = load_w("wv_i")
        for tch in range(2, NCH):
            load_x_chunk(tch)

        ones_st = const.tile([P, P], F32)
        nc.vector.memset(ones_st, 1.0)
        ones_col = const.tile([P, 1], BF16)   # lhsT for l = ones^T @ expS
        nc.scalar.activation(ones_col, ones_st[:, 0:1],
                             mybir.ActivationFunctionType.Copy)
        ones_row = const.tile([1, P], BF16)   # lhsT for K=1 broadcasts
        nc.scalar.activation(ones_row, ones_st[0:1, :],
                             mybir.ActivationFunctionType.Copy)
        # Causal mask tiles: mask[o][p, f] = (f - p >= 128*o), o = k - 4j
        masks = const.tile([P, QKB, QCH], BF16)
        nc.vector.memset(masks, 1.0)
        for o in range(QKB):
            nc.gpsimd.affine_select(
                out=masks[:, o, :], in_=masks[:, o, :],
                compare_op=mybir.AluOpType.is_ge, fill=0.0,
                base=-P * o, pattern=[[1, QCH]], channel_multiplier=-1)

        # Q/K resident, complex-stacked per head (odd heads swapped)
        qh = [opool.tile([P, T], BF16, name=f"qh{h}") for h in range(HPC)]
        kh = [opool.tile([P, T], BF16, name=f"kh{h}") for h in range(HPC)]
        # V resident: [p, ktile, head*128 + (64|64)]; even head [vr|vi],
        # odd head [vi|vr]
        v_sb = opool.tile([P, KT, HPC * P], BF16)
        # O^T head-pair blocks, SBUF-resident into phase 3.
        # ORT[pair] rows: [vr_h_even(64) ; vr_h_odd(64)]
        # OIT[pair] rows: [vi_h_odd(64) ; vi_h_even(64)]  (host permutes wo_i)
        ort = [opool.tile([P, T], BF16, name=f"ort{p}") for p in range(NPAIR)]
        oit = [opool.tile([P, T], BF16, name=f"oit{p}") for p in range(NPAIR)]

        # ================= Phase 1: projections =================
        with ExitStack() as ctx:
            ps_qk = ctx.enter_context(tc.tile_pool(name="ps_qk", bufs=4, space="PSUM"))

            # Q/K pair-packed: psA = [re_h0;re_h1], psB = [im_h1;im_h0]
            def qk_chunk(tch):
                cs = slice(tch * TCH, (tch + 1) * TCH)
                for pr in range(NPAIR):
                    h0, h1 = 2 * pr, 2 * pr + 1
                    mA = slice(pr * P, pr * P + P)
                    for (wr, wi, bias, dst) in (
                        (wq_r_sb, wq_i_sb, bq_sb, qh),
                        (wk_r_sb, wk_i_sb, bk_sb, kh),
                    ):
                        psA = ps_qk.tile([P, TCH], F32, name="psA", tag="psqk")
                        psB = ps_qk.tile([P, TCH], F32, name="psB", tag="psqk")
                        for d in range(DT):
                            nc.tensor.matmul(
                                psA, wr[:, d, mA], xrT[:, d, cs],
                                start=(d == 0), stop=(d == DT - 1))
                            # host swapped the imag pair columns -> [h1, h0]
                            nc.tensor.matmul(
                                psB, wi[:, d, mA], xiT[:, d, cs],
                                start=(d == 0), stop=(d == DT - 1))
                        nc.vector.tensor_scalar_add(
                            out=dst[h0][0:64, cs], in0=psA[0:64],
                            scalar1=bias[0:64, h0:h0 + 1])
                        nc.vector.tensor_scalar_add(
                            out=dst[h1][64:128, cs], in0=psA[64:128],
                            scalar1=bias[64:128, h1:h1 + 1])
                        nc.vector.tensor_scalar_add(
                            out=dst[h1][0:64, cs], in0=psB[0:64],
                            scalar1=bias[0:64, h1:h1 + 1])
                        nc.vector.tensor_scalar_add(
                            out=dst[h0][64:128, cs], in0=psB[64:128],
                            scalar1=bias[64:128, h0:h0 + 1])

            # V token-major: psum [tok(128), CW] for r and i, then pack into
            # v_sb[:, kt, head*128 + ...] with odd-head [vi|vr] swap.
            # Pack copies go on ACT (idle during phase 1 otherwise).
            ps_v1 = ctx.enter_context(tc.tile_pool(name="ps_v1", bufs=4, space="PSUM"))

            def v_ktile(ktile):
                t0 = ktile * P
                pvr = ps_v1.tile([P, CW], F32, name="pvr", tag="pv")
                for d in range(DT):
                    nc.tensor.matmul(
                        pvr, xrT[:, d, t0:t0 + P], wv_r_sb[:, d, :],
                        start=(d == 0), stop=(d == DT - 1))
                pvi = ps_v1.tile([P, CW], F32, name="pvi", tag="pv")
                for d in range(DT):
                    nc.tensor.matmul(
                        pvi, xiT[:, d, t0:t0 + P], wv_i_sb[:, d, :],
                        start=(d == 0), stop=(d == DT - 1))
                for h in range(HPC):
                    lo = h % 2
                    dst_r = v_sb[:, ktile,
                                 h * P + 64 * lo:h * P + 64 * lo + 64]
                    dst_i = v_sb[:, ktile,
                                 h * P + 64 * (1 - lo):h * P + 64 * (1 - lo) + 64]
                    nc.scalar.copy(out=dst_r, in_=pvr[:, h * DH:(h + 1) * DH])
                    nc.scalar.copy(out=dst_i, in_=pvi[:, h * DH:(h + 1) * DH])

            # Interleave: V matmul stretches give the DVE add queue time to
            # drain before phase 2 consumes the last chunks' Q/K.
            qk_chunk(0)
            qk_chunk(1)
            for kt_ in range(0, KT // 2):
                v_ktile(kt_)
            qk_chunk(2)
            qk_chunk(3)
            for kt_ in range(KT // 2, KT):
                v_ktile(kt_)

        # ========== Phase 2+3: causal attention + output projection ==========
        # j-outer: after all 4 heads finish query-chunk j, the phase-3
        # projection for that chunk's 4 token tiles is emitted.
        with ExitStack() as ctx:
            epool = ctx.enter_context(tc.tile_pool(name="epool", bufs=12))
            rpool = ctx.enter_context(tc.tile_pool(name="rpool", bufs=2))
            sout = ctx.enter_context(tc.tile_pool(name="sout", bufs=6))
            # 8 PSUM banks: big(st,pb,pf) 3 + po 3 + pl 2
            ps_big = ctx.enter_context(tc.tile_pool(name="ps_big", bufs=3, space="PSUM"))
            ps_o = ctx.enter_context(tc.tile_pool(name="ps_o", bufs=3, space="PSUM"))
            ps_l = ctx.enter_context(tc.tile_pool(name="ps_l", bufs=2, space="PSUM"))

            wor_sb = wpool.tile([P, NPAIR, D], BF16, name="wor_sb")
            nc.sync.dma_start(wor_sb, bslice("wo_r", (NPAIR, D)))
            woi_sb = wpool.tile([P, NPAIR, D], BF16, name="woi_sb")
            nc.sync.dma_start(woi_sb, bslice("wo_i", (NPAIR, D)))

            NC2 = D // 512
            flip = 0

            def phase3_chunk(j, parts=(0, 1)):
                nonlocal flip
                tensors = ((ort, wor_sb, 0), (oit, woi_sb, 1))
                for t in range(j * QKB, (j + 1) * QKB):
                    for (oblocks, wsb, ri) in (tensors[p] for p in parts):
                        for n in range(NC2):
                            pf = ps_big.tile([P, 512], F32, name="pf",
                                             tag="big")
                            for kk in range(NPAIR):
                                nc.tensor.matmul(
                                    pf,
                                    oblocks[kk][:, t * P:(t + 1) * P],
                                    wsb[:, kk, n * 512:(n + 1) * 512],
                                    start=(kk == 0), stop=(kk == NPAIR - 1))
                            ot = sout.tile([P, 512], BF16, name="ot")
                            # alternate the cast-copy between ACT and DVE
                            # (GPSIMD cannot read PSUM)
                            if flip % 2 == 0:
                                nc.scalar.copy(out=ot, in_=pf)
                            else:
                                nc.vector.tensor_copy(out=ot, in_=pf)
                            flip += 1
                            nc.sync.dma_start(
                                out_t[:, t, ri, n * 512:(n + 1) * 512], ot)

            # Flat block list; the score->exp->mask stage cursor runs two
            # k-steps ahead GLOBALLY (crossing block boundaries), so the PE
            # never waits for the ACT round trip, even at block starts.
            blocks = [(j, pr) for j in range(QC) for pr in range(NPAIR)]
            ets = {}

            def stage(bi, k):
                j, pr = blocks[bi]
                # Diagonal blocks (o >= 1): columns f < 128*o are fully
                # masked -> compute st/exp/mask/pl/po on [128*o, QCH) only.
                o = k - j * QKB
                off = max(0, o) * P if o >= 1 else 0
                qs = slice(j * QCH + off, (j + 1) * QCH)
                for h in (2 * pr, 2 * pr + 1):
                    st = ps_big.tile([P, QCH], F32, name="st", tag="big")
                    nc.tensor.matmul(
                        st[:, off:], kh[h][:, k * P:(k + 1) * P],
                        qh[h][:, qs], start=True, stop=True)
                    et = epool.tile([P, QCH], BF16, name="et")
                    nc.scalar.activation(
                        et[:, off:], st[:, off:],
                        mybir.ActivationFunctionType.Exp, scale=scale)
                    if o >= 0:
                        nc.vector.tensor_mul(
                            out=et[:, off:], in0=et[:, off:],
                            in1=masks[:, o, off:])
                    ets[bi, h, k] = et

            cursor = [0, 0]

            def emit_next_stage():
                bi, k = cursor
                if bi >= len(blocks):
                    return
                stage(bi, k)
                if k + 1 < (blocks[bi][0] + 1) * QKB:
                    cursor[:] = [bi, k + 1]
                else:
                    cursor[:] = [bi + 1, 0]

            for _ in range(5):
                emit_next_stage()
            for bi, (j, pr) in enumerate(blocks):
                nk = (j + 1) * QKB
                qs = slice(j * QCH, (j + 1) * QCH)
                hs = (2 * pr, 2 * pr + 1)
                # Dependency-free phase-3 PE filler at block transitions
                # (chunk j's ort/oit complete after block (j, pair1)).
                filler = {2: lambda: phase3_chunk(0, parts=(0,)),
                          3: lambda: phase3_chunk(0, parts=(1,)),
                          4: lambda: phase3_chunk(1, parts=(0,)),
                          5: lambda: phase3_chunk(1, parts=(1,)),
                          6: lambda: phase3_chunk(2, parts=(0,)),
                          7: lambda: phase3_chunk(2, parts=(1,))}.get(bi)
                if filler is not None:
                    filler()
                po = {h: ps_o.tile([P, QCH], F32, name=f"po{h}", tag="po")
                      for h in hs}
                pl = {h: ps_l.tile([1, QCH], F32, name=f"pl{h}", tag="pl")
                      for h in hs}
                for k in range(nk):
                    emit_next_stage()
                    o = k - j * QKB
                    off = max(0, o) * P if o >= 1 else 0
                    for h in hs:
                        et = ets.pop((bi, h, k))
                        nc.tensor.matmul(
                            pl[h][:, off:], ones_col, et[:, off:],
                            start=(k == 0), stop=(k == nk - 1))
                        # po rows: even head [o_r;o_i], odd [o_i;o_r]
                        nc.tensor.matmul(
                            po[h][:, off:], v_sb[:, k, h * P:(h + 1) * P],
                            et[:, off:],
                            start=(k == 0), stop=(k == nk - 1))

                rls = {}
                for h in hs:
                    rl = rpool.tile([1, QCH], BF16, name="rl")
                    with nc.allow_low_precision(
                            reason="1/l in bf16 feeds bf16 bcast matmul"):
                        nc.vector.reciprocal(rl, pl[h])
                    rls[h] = rl
                for h in hs:
                    lo = h % 2
                    base_r = 64 * lo          # vr rows in ORT[pair]
                    base_i = 64 * (1 - lo)    # vi rows in OIT[pair]
                    pb = ps_big.tile([P, QCH], F32, name="pb", tag="big")
                    nc.tensor.matmul(pb, ones_row, rls[h],
                                     start=True, stop=True)
                    sb_b = rpool.tile([P, QCH], BF16, name="sb_b")
                    nc.vector.tensor_copy(out=sb_b, in_=pb)
                    nc.vector.tensor_mul(
                        out=ort[pr][base_r:base_r + 64, qs],
                        in0=po[h][64 * lo:64 * lo + 64],
                        in1=sb_b[64 * lo:64 * lo + 64])
                    nc.vector.tensor_mul(
                        out=oit[pr][base_i:base_i + 64, qs],
                        in0=po[h][64 * (1 - lo):64 * (1 - lo) + 64],
                        in1=sb_b[64 * (1 - lo):64 * (1 - lo) + 64])
            phase3_chunk(QC - 1)

    nc.compile()
    return nc


def make_core_inputs(inputs, cfg=CFG):
    """Pack full inputs into 8 per-core single-blob maps (bf16, SBUF
    layout, pre-transposed x, pair-swapped imag weight columns,
    odd-head-swapped biases)."""
    T, D, HPC, DH = cfg["T"], cfg["D"], cfg["HPC"], cfg["DH"]
    CW = HPC * DH
    DT = D // P
    NPAIR = HPC // 2
    offs, cols = _blob_offsets(cfg)
    f32 = lambda a: np.asarray(a, dtype=np.float32)

    def sb_layout(w, t):
        # [t*128, m] -> [128, t, m] -> [128, t*m]   (partition-major)
        m = w.shape[1]
        return np.ascontiguousarray(
            w.reshape(t, P, m).transpose(1, 0, 2).reshape(P, t * m))

    def pair_swap_cols(w):
        out = np.empty_like(w)
        for p in range(NPAIR):
            out[:, p * 2 * DH:p * 2 * DH + DH] = \
                w[:, p * 2 * DH + DH:p * 2 * DH + 2 * DH]
            out[:, p * 2 * DH + DH:p * 2 * DH + 2 * DH] = \
                w[:, p * 2 * DH:p * 2 * DH + DH]
        return out

    x_real, x_imag = f32(inputs["x_real"]), f32(inputs["x_imag"])
    maps = []
    for c in range(N_CORES):
        b = c // 4
        g = c % 4
        cs = slice(g * CW, (g + 1) * CW)
        bqr, bqi = f32(inputs["bqr"])[cs], f32(inputs["bqi"])[cs]
        bkr, bki = f32(inputs["bkr"])[cs], f32(inputs["bki"])[cs]

        def head_bias(br, bi):
            # col h: even [br_h; bi_h], odd [bi_h; br_h]
            cols_ = []
            for h in range(HPC):
                r = br[h * DH:(h + 1) * DH]
                i = bi[h * DH:(h + 1) * DH]
                cols_.append(np.concatenate([r, i] if h % 2 == 0 else [i, r]))
            return np.ascontiguousarray(np.stack(cols_, axis=1))

        woi = f32(inputs["Woi"])[cs, :]
        woi_perm = np.concatenate(
            [np.concatenate([woi[2 * p * DH + DH:2 * p * DH + 2 * DH],
                             woi[2 * p * DH:2 * p * DH + DH]])
             for p in range(NPAIR)])
        parts = {
            "xt_r": sb_layout(x_real[b].T, DT),
            "xt_i": sb_layout(x_imag[b].T, DT),
            "wq_r": sb_layout(f32(inputs["Wqr"])[:, cs], DT),
            "wq_i": sb_layout(pair_swap_cols(f32(inputs["Wqi"])[:, cs]), DT),
            "wk_r": sb_layout(f32(inputs["Wkr"])[:, cs], DT),
            "wk_i": sb_layout(pair_swap_cols(f32(inputs["Wki"])[:, cs]), DT),
            "wv_r": sb_layout(f32(inputs["Wvr"])[:, cs], DT),
            "wv_i": sb_layout(f32(inputs["Wvi"])[:, cs], DT),
            "wo_r": sb_layout(f32(inputs["Wor"])[cs, :], NPAIR),
            "wo_i": sb_layout(woi_perm, NPAIR),
            "bq": head_bias(bqr, bqi),
            "bk": head_bias(bkr, bki),
        }
        blob = np.zeros((P, cols), np.float32)
        for nm, arr in parts.items():
            o, sz = offs[nm]
            blob[:, o:o + sz] = arr
        maps.append({"blob": blob.astype(NP_BF16)})
    return maps


def effective_out_bias(inputs):
    """V bias folded through the output projection (softmax weights sum
    to 1): bor_eff = bor + bvr @ Wor, boi_eff = boi + bvi @ Woi."""
    f = lambda a: np.asarray(a, dtype=np.float32)
    bor = f(inputs["bor"]) + f(inputs["bvr"]) @ f(inputs["Wor"])
    boi = f(inputs["boi"]) + f(inputs["bvi"]) @ f(inputs["Woi"])
    return bor, boi


def assemble_output(results, inputs):
    """Sum the 4 head-group partials per batch, split r/i, add bias."""
    f = lambda a: np.asarray(a, dtype=np.float32)
    bor, boi = effective_out_bias(inputs)
    final_r = np.stack([
        sum(f(results[c]["out"][:, 0]) for c in range(4 * b, 4 * b + 4)) + bor
        for b in range(B)]).astype(np.float32)
    final_i = np.stack([
        sum(f(results[c]["out"][:, 1]) for c in range(4 * b, 4 * b + 4)) + boi
        for b in range(B)]).astype(np.float32)
    return final_r, final_i


def kernel(**inputs):
    global LAST
    nc = build_program(CFG)
    in_maps = make_core_inputs(inputs)
    res = run_bass_kernel_spmd(
        nc, in_maps, core_ids=list(range(N_CORES)), trace=TRACE)
    LAST = {"exec_time_ns": res.exec_time_ns,
            "trace": res.instructions_and_trace,
            "profile_json": res.profile_json,
            "nc": nc}
    results = [{k: np.asarray(v) for k, v in res.results[c].items()}
               for c in range(N_CORES)]
    return assemble_output(results, inputs)
